# revision 1
# baseline (speedup 1.0000x reference)
"""DetectionLoss Bass kernel for TRN2, 8-core SPMD.

Strategy:
- Device (identical program on all 8 cores; inputs differ only in the
  vocab slice of caption_logits):
  * build the (64,256) fused cost matrix (both samples stacked on the
    partition dim) from boxes + objectness,
  * run the 32-step greedy matching on the vector engine (per-row top-1
    via max/max_index, 32x32 stream transpose, per-sample argmax,
    dynamic-offset masking via registers),
  * per step, indirect-DMA-gather only the matched prediction's caption
    logit rows (30 rows of V/8 floats) - overlapping the big gather with
    the serial matching,
  * exp + free-dim accumulate on ACT -> per-(b,step,pos) partial sum(exp)
    over this core's vocab slice,
  * matched-pair L1/GIoU bbox loss and objectness BCE reduced to
    per-sample scalars on device.
- Host: shards caption_logits by vocab (plus small layout prep /
  broadcast of the box rows), all-reduces the per-core partial sumexps,
  takes log, gathers target-token logits, and combines the scalar
  losses (the final weighted mean).
"""

import sys

sys.path.insert(0, "/opt/trn_rl_repo")

import numpy as np

import concourse.bacc as bacc
import concourse.mybir as mybir
from concourse.bass import ds
from concourse.tile import TileContext

F32 = mybir.dt.float32
I32 = mybir.dt.int32
U32 = mybir.dt.uint32
Alu = mybir.AluOpType
Act = mybir.ActivationFunctionType

B, N, M, L = 2, 256, 32, 16
LM1 = L - 1  # 15 caption positions
S = M  # greedy steps
NEG = -1.0e9
EPS = 1e-7
ROWS_PER_STEP = B * LM1  # 30 gathered rows per step
STEPS_PER_BATCH = 4
NBATCH = S // STEPS_PER_BATCH  # 8 ACT sweeps over (120, V8)
GP = STEPS_PER_BATCH * ROWS_PER_STEP  # 120


def build_nc(V8: int, num_devices: int = 8):
    """Build the per-core Bass program. V8 = vocab slice width per core."""
    nc = bacc.Bacc(
        "TRN2", target_bir_lowering=False, debug=False, num_devices=num_devices
    )
    DVE = (mybir.EngineType.DVE,)
    DVESP = (mybir.EngineType.DVE, mybir.EngineType.SP)
    DVEACT = (mybir.EngineType.DVE, mybir.EngineType.SP)

    cl = nc.dram_tensor("cl", (B * N * L, V8), F32, kind="ExternalInput")
    # pbig: per (b,j) partition, 9 x 256 row segments:
    # [x1n y1n x2n y2n x1 y1 x2 y2 po]
    pbig = nc.dram_tensor("pbig", (64, 9 * N), F32, kind="ExternalInput")
    po = nc.dram_tensor("po", (B * N, 1), F32, kind="ExternalInput")
    pb = nc.dram_tensor("pb", (B * N, 4), F32, kind="ExternalInput")
    gb = nc.dram_tensor("gb", (B * M, 4), F32, kind="ExternalInput")
    out = nc.dram_tensor("out", (128, 16), F32, kind="ExternalOutput")

    # per-sample DRAM views for register-offset gathers
    cl2 = cl[:].rearrange("(b n l) v -> b n (l v)", b=B, n=N)  # (2, 256, L*V8)
    pbv = pb[:].rearrange("(b n) c -> b n c", b=B)
    gbv = gb[:].rearrange("(b m) c -> b m c", b=B)
    pov = po[:].rearrange("(b n) o -> b n o", b=B)

    with TileContext(nc) as tc:
        with (
            tc.tile_pool(name="cpool", bufs=1) as cp,
            tc.tile_pool(name="opool", bufs=4) as op,
            tc.tile_pool(name="gpool", bufs=3) as gp,
            tc.tile_pool(name="dpool", bufs=1) as dp,
        ):
            # ---------- input loads ----------
            pbig_sb = cp.tile([64, 9 * N], F32)
            nc.sync.dma_start(pbig_sb[:], pbig[:])

            def seg(k):
                return pbig_sb[:, k * N : (k + 1) * N]

            gb_sb = cp.tile([64, 4], F32)
            nc.sync.dma_start(gb_sb[:], gb[:])

            ts = nc.vector.tensor_scalar
            tt = nc.vector.tensor_tensor

            # ---------- cost matrix build ----------
            # gt cols (64,1)
            gx1n = cp.tile([64, 1], F32)
            gy1n = cp.tile([64, 1], F32)
            gx2n = cp.tile([64, 1], F32)
            gy2n = cp.tile([64, 1], F32)
            nc.vector.tensor_tensor(gx1n[:], gb_sb[:, 0:1], gb_sb[:, 2:3], op=Alu.min)
            nc.vector.tensor_tensor(gx2n[:], gb_sb[:, 0:1], gb_sb[:, 2:3], op=Alu.max)
            nc.vector.tensor_tensor(gy1n[:], gb_sb[:, 1:2], gb_sb[:, 3:4], op=Alu.min)
            nc.vector.tensor_tensor(gy2n[:], gb_sb[:, 1:2], gb_sb[:, 3:4], op=Alu.max)
            ga2 = cp.tile([64, 1], F32)
            gw = cp.tile([64, 1], F32)
            gh = cp.tile([64, 1], F32)
            nc.vector.tensor_tensor(gw[:], gx2n[:], gx1n[:], op=Alu.subtract)
            nc.vector.tensor_tensor(gh[:], gy2n[:], gy1n[:], op=Alu.subtract)
            nc.vector.tensor_tensor(ga2[:], gw[:], gh[:], op=Alu.mult)

            xi1 = cp.tile([64, N], F32)
            xi2 = cp.tile([64, N], F32)
            xe1 = cp.tile([64, N], F32)
            xe2 = cp.tile([64, N], F32)
            ts(xi1[:], seg(0), gx1n[:], None, op0=Alu.max)
            ts(xi2[:], seg(2), gx2n[:], None, op0=Alu.min)
            ts(xe1[:], seg(0), gx1n[:], None, op0=Alu.min)
            ts(xe2[:], seg(2), gx2n[:], None, op0=Alu.max)
            yi1 = cp.tile([64, N], F32)
            yi2 = cp.tile([64, N], F32)
            ye1 = cp.tile([64, N], F32)
            ye2 = cp.tile([64, N], F32)
            ts(yi1[:], seg(1), gy1n[:], None, op0=Alu.max)
            ts(yi2[:], seg(3), gy2n[:], None, op0=Alu.min)
            ts(ye1[:], seg(1), gy1n[:], None, op0=Alu.min)
            ts(ye2[:], seg(3), gy2n[:], None, op0=Alu.max)

            iw = cp.tile([64, N], F32)
            ih = cp.tile([64, N], F32)
            tt(iw[:], xi2[:], xi1[:], op=Alu.subtract)
            ts(iw[:], iw[:], 0.0, None, op0=Alu.max)
            tt(ih[:], yi2[:], yi1[:], op=Alu.subtract)
            ts(ih[:], ih[:], 0.0, None, op0=Alu.max)
            inter = cp.tile([64, N], F32)
            tt(inter[:], iw[:], ih[:], op=Alu.mult)

            ew = cp.tile([64, N], F32)
            eh = cp.tile([64, N], F32)
            tt(ew[:], xe2[:], xe1[:], op=Alu.subtract)
            tt(eh[:], ye2[:], ye1[:], op=Alu.subtract)
            enc = cp.tile([64, N], F32)
            tt(enc[:], ew[:], eh[:], op=Alu.mult)

            # a1 = (x2n-x1n)*(y2n-y1n); union = a1 + a2 - inter
            a1 = cp.tile([64, N], F32)
            a1h = cp.tile([64, N], F32)
            tt(a1[:], seg(2), seg(0), op=Alu.subtract)
            tt(a1h[:], seg(3), seg(1), op=Alu.subtract)
            tt(a1[:], a1[:], a1h[:], op=Alu.mult)
            union = cp.tile([64, N], F32)
            ts(union[:], a1[:], ga2[:], None, op0=Alu.add)
            tt(union[:], union[:], inter[:], op=Alu.subtract)

            iou = cp.tile([64, N], F32)
            tmp = cp.tile([64, N], F32)
            ts(tmp[:], union[:], EPS, None, op0=Alu.add)
            nc.vector.reciprocal(tmp[:], tmp[:])
            tt(iou[:], inter[:], tmp[:], op=Alu.mult)

            # giou = iou - (enc - union)/(enc + eps)
            giou = cp.tile([64, N], F32)
            tt(giou[:], enc[:], union[:], op=Alu.subtract)
            ts(tmp[:], enc[:], EPS, None, op0=Alu.add)
            nc.vector.reciprocal(tmp[:], tmp[:])
            tt(giou[:], giou[:], tmp[:], op=Alu.mult)
            tt(giou[:], iou[:], giou[:], op=Alu.subtract)

            # l1 from raw comps (segments 4..7)
            l1s = cp.tile([64, N], F32)
            dc = cp.tile([64, N], F32)
            for c in range(4):
                dst = l1s if c == 0 else dc
                ts(dst[:], seg(4 + c), gb_sb[:, c : c + 1], None,
                   op0=Alu.subtract)
                nc.scalar.activation(dst[:], dst[:], Act.Abs)
                if c > 0:
                    tt(l1s[:], l1s[:], dc[:], op=Alu.add)

            # objectness term: sigmoid(po) - 2 (po broadcast = segment 8)
            # sigmoid(x) = 1/(1+exp(-x)); only Exp/Ln/Identity share one
            # ACT table, so avoid Sigmoid/Softplus entirely.
            sig2 = cp.tile([64, N], F32)
            nc.scalar.activation(sig2[:], seg(8), Act.Exp, scale=-1.0)
            ts(sig2[:], sig2[:], 1.0, None, op0=Alu.add)
            nc.vector.reciprocal(sig2[:], sig2[:])
            ts(sig2[:], sig2[:], -2.0, None, op0=Alu.add)

            ncf = cp.tile([64, N], F32)
            tt(ncf[:], giou[:], l1s[:], op=Alu.subtract)
            tt(ncf[:], ncf[:], sig2[:], op=Alu.add)
            # split per-sample so dynamic-offset masking stays at partition 0
            ncA = cp.tile([64, N], F32)
            ncB = cp.tile([64, N], F32)
            nc.vector.tensor_copy(ncA[0:32, :], ncf[0:32, :])
            nc.vector.tensor_copy(ncB[0:32, :], ncf[32:64, :])

            # ---------- greedy matching ----------
            # Partition-layout rule: every compute AP must start at
            # partition 0/32/64/96. Sample b0 data lives at partition 0,
            # sample b1 at partition 32, throughout.
            # fully per-sample tile sets; every compute AP starts at
            # partition 0 (NCC_IBIR297: two-SB-input ops need equal bases)
            pkA = cp.tile([64, 32], F32)
            nc.vector.memset(pkA[:], 0.0)
            pkB = cp.tile([64, 32], F32)
            nc.vector.memset(pkB[:], 0.0)
            pk2A = cp.tile([64, 32], F32)
            nc.vector.memset(pk2A[:], 0.0)
            pk2B = cp.tile([64, 32], F32)
            nc.vector.memset(pk2B[:], 0.0)
            pkTA = cp.tile([64, 32], F32)
            pkTB = cp.tile([64, 32], F32)
            pk2TA = cp.tile([64, 32], F32)
            pk2TB = cp.tile([64, 32], F32)
            ridxA = cp.tile([64, 8], U32)
            ridxB = cp.tile([64, 8], U32)
            tmA = cp.tile([64, 32], F32)
            tmB = cp.tile([64, 32], F32)
            g8A = cp.tile([64, 8], F32)
            g8B = cp.tile([64, 8], F32)
            giA = cp.tile([64, 8], U32)
            giB = cp.tile([64, 8], U32)
            gtmA = cp.tile([64, 32], F32)
            nc.vector.memset(gtmA[:], 0.0)
            gtmB = cp.tile([64, 32], F32)
            nc.vector.memset(gtmB[:], 0.0)
            pisr = cp.tile([64, 32], F32)  # row0 = pis b0, row32 = pis b1
            nc.vector.memset(pisr[:], 0.0)
            gjsr = cp.tile([64, 32], F32)
            nc.vector.memset(gjsr[:], 0.0)
            pisri = cp.tile([64, 32], I32)  # write-once per-step int columns
            gjsri = cp.tile([64, 32], I32)

            outsb = cp.tile([128, 16], F32)
            nc.vector.memset(outsb[:], 0.0)
            mp = cp.tile([64, 4], F32)
            mg = cp.tile([64, 4], F32)
            pom = cp.tile([64, 1], F32)

            for s in range(S):
                nc.vector.max(pkA[0:32, 0:8], ncA[0:32, :])
                nc.vector.max_index(ridxA[0:32], pkA[0:32, 0:8], ncA[0:32, :])
                nc.vector.max(pkB[0:32, 0:8], ncB[0:32, :])
                nc.vector.max_index(ridxB[0:32], pkB[0:32, 0:8], ncB[0:32, :])
                nc.vector.tensor_copy(pk2A[0:32, 0:1], ridxA[0:32, 0:1])
                nc.vector.tensor_copy(pk2B[0:32, 0:1], ridxB[0:32, 0:1])
                nc.vector.transpose(pkTA[0:32, :], pkA[0:32, :])
                nc.vector.transpose(pkTB[0:32, :], pkB[0:32, :])
                nc.vector.transpose(pk2TA[0:32, :], pk2A[0:32, :])
                nc.vector.transpose(pk2TB[0:32, :], pk2B[0:32, :])
                tt(tmA[0:1], pkTA[0:1, 0:32], gtmA[0:1], op=Alu.add)
                tt(tmB[0:1], pkTB[0:1, 0:32], gtmB[0:1], op=Alu.add)
                nc.vector.max(g8A[0:1], tmA[0:1])
                nc.vector.max_index(giA[0:1], g8A[0:1], tmA[0:1])
                nc.vector.max(g8B[0:1], tmB[0:1])
                nc.vector.max_index(giB[0:1], g8B[0:1], tmB[0:1])
                nc.vector.tensor_copy(gjsri[0:1, s : s + 1], giA[0:1, 0:1])
                nc.vector.tensor_copy(gjsri[32:33, s : s + 1], giB[0:1, 0:1])
                j0 = nc.values_load(gjsri[0:1, s : s + 1], engines=DVESP,
                                    min_val=0, max_val=31,
                                    skip_runtime_bounds_check=True)
                j1 = nc.values_load(gjsri[32:33, s : s + 1], engines=DVESP,
                                    min_val=0, max_val=31,
                                    skip_runtime_bounds_check=True)
                i_f = op.tile([64, 1], F32, tag="i_f")
                nc.vector.tensor_copy(i_f[0:1], pk2TA[0:1, ds(j0, 1)])
                nc.vector.tensor_copy(i_f[32:33], pk2TB[0:1, ds(j1, 1)])
                nc.vector.tensor_copy(pisri[0:1, s : s + 1], i_f[0:1])
                nc.vector.tensor_copy(pisri[32:33, s : s + 1], i_f[32:33])
                i0 = nc.values_load(pisri[0:1, s : s + 1], engines=DVESP,
                                    min_val=0, max_val=N - 1,
                                    skip_runtime_bounds_check=True)
                i1 = nc.values_load(pisri[32:33, s : s + 1], engines=DVESP,
                                    min_val=0, max_val=N - 1,
                                    skip_runtime_bounds_check=True)
                nc.vector.memset(ncA[0:32, ds(i0, 1)], NEG)
                nc.vector.memset(ncB[0:32, ds(i1, 1)], NEG)
                nc.vector.memset(gtmA[0:1, ds(j0, 1)], NEG)
                nc.vector.memset(gtmB[0:1, ds(j1, 1)], NEG)
                nc.vector.tensor_copy(pisr[0:1, s : s + 1], i_f[0:1])
                nc.vector.tensor_copy(pisr[32:33, s : s + 1], i_f[32:33])
                nc.vector.tensor_copy(gjsr[0:1, s : s + 1], giA[0:1, 0:1])
                nc.vector.tensor_copy(gjsr[32:33, s : s + 1], giB[0:1, 0:1])

                # caption logit rows of the two matched preds: contiguous
                # (L-1)*V8 slabs fetched with register-offset DMAs (HWDGE).
                g, k = divmod(s, STEPS_PER_BATCH)
                if k == 0:
                    gtile = gp.tile([128, V8], F32, tag="gtile")
                base = k * ROWS_PER_STEP
                nc.sync.dma_start(
                    gtile[base : base + LM1, :],
                    cl2[0, ds(i0, 1), 0 : LM1 * V8])
                nc.sync.dma_start(
                    gtile[base + LM1 : base + ROWS_PER_STEP, :],
                    cl2[1, ds(i1, 1), 0 : LM1 * V8])
                # matched boxes / objectness, one row per step per sample
                nc.sync.dma_start(mp[s : s + 1, :], pbv[0, ds(i0, 1), :])
                nc.sync.dma_start(mp[32 + s : 33 + s, :], pbv[1, ds(i1, 1), :])
                nc.sync.dma_start(mg[s : s + 1, :], gbv[0, ds(j0, 1), :])
                nc.sync.dma_start(mg[32 + s : 33 + s, :], gbv[1, ds(j1, 1), :])
                nc.sync.dma_start(pom[s : s + 1, :], pov[0, ds(i0, 1), :])
                nc.sync.dma_start(pom[32 + s : 33 + s, :], pov[1, ds(i1, 1), :])
                if k == STEPS_PER_BATCH - 1:
                    dump = dp.tile([128, V8], F32, tag="dump")
                    nc.scalar.activation(dump[0:GP, :], gtile[0:GP, :], Act.Exp,
                                         accum_out=outsb[0:GP, g : g + 1])

            # ---------- post: pis/gjs columns via stream transpose ----------
            pgT = cp.tile([64, 32], F32)
            ggT = cp.tile([64, 32], F32)
            nc.vector.transpose(pgT[:], pisr[:])
            nc.vector.transpose(ggT[:], gjsr[:])
            # pgT[0:32,0] = pis b0; pgT[32:64,0] = pis b1
            nc.vector.tensor_copy(outsb[0:32, 8:9], pgT[0:32, 0:1])
            nc.vector.tensor_copy(outsb[32:64, 8:9], pgT[32:64, 0:1])
            nc.vector.tensor_copy(outsb[0:32, 9:10], ggT[0:32, 0:1])
            nc.vector.tensor_copy(outsb[32:64, 9:10], ggT[32:64, 0:1])

            # ---------- matched-pair bbox loss ----------
            md = cp.tile([64, 4], F32)
            l1p = cp.tile([64, 1], F32)
            tt(md[:], mp[:], mg[:], op=Alu.subtract)
            nc.scalar.activation(md[:], md[:], Act.Abs, accum_out=l1p[:])

            def col(t, c):
                return t[:, c : c + 1]

            mx1 = cp.tile([64, 1], F32)
            my1 = cp.tile([64, 1], F32)
            mx2 = cp.tile([64, 1], F32)
            my2 = cp.tile([64, 1], F32)
            tt(mx1[:], col(mp, 0), col(mp, 2), op=Alu.min)
            tt(mx2[:], col(mp, 0), col(mp, 2), op=Alu.max)
            tt(my1[:], col(mp, 1), col(mp, 3), op=Alu.min)
            tt(my2[:], col(mp, 1), col(mp, 3), op=Alu.max)
            nx1 = cp.tile([64, 1], F32)
            ny1 = cp.tile([64, 1], F32)
            nx2 = cp.tile([64, 1], F32)
            ny2 = cp.tile([64, 1], F32)
            tt(nx1[:], col(mg, 0), col(mg, 2), op=Alu.min)
            tt(nx2[:], col(mg, 0), col(mg, 2), op=Alu.max)
            tt(ny1[:], col(mg, 1), col(mg, 3), op=Alu.min)
            tt(ny2[:], col(mg, 1), col(mg, 3), op=Alu.max)

            w1 = cp.tile([64, 1], F32)
            w2 = cp.tile([64, 1], F32)
            w3 = cp.tile([64, 1], F32)
            w4 = cp.tile([64, 1], F32)
            tt(w1[:], mx1[:], nx1[:], op=Alu.max)  # xi1
            tt(w2[:], mx2[:], nx2[:], op=Alu.min)  # xi2
            tt(w2[:], w2[:], w1[:], op=Alu.subtract)
            ts(w2[:], w2[:], 0.0, None, op0=Alu.max)  # iw
            tt(w1[:], my1[:], ny1[:], op=Alu.max)
            tt(w3[:], my2[:], ny2[:], op=Alu.min)
            tt(w3[:], w3[:], w1[:], op=Alu.subtract)
            ts(w3[:], w3[:], 0.0, None, op0=Alu.max)  # ih
            minter = cp.tile([64, 1], F32)
            tt(minter[:], w2[:], w3[:], op=Alu.mult)
            tt(w1[:], mx2[:], mx1[:], op=Alu.subtract)
            tt(w2[:], my2[:], my1[:], op=Alu.subtract)
            tt(w1[:], w1[:], w2[:], op=Alu.mult)  # a1
            tt(w2[:], nx2[:], nx1[:], op=Alu.subtract)
            tt(w3[:], ny2[:], ny1[:], op=Alu.subtract)
            tt(w2[:], w2[:], w3[:], op=Alu.mult)  # a2
            munion = cp.tile([64, 1], F32)
            tt(munion[:], w1[:], w2[:], op=Alu.add)
            tt(munion[:], munion[:], minter[:], op=Alu.subtract)
            miou = cp.tile([64, 1], F32)
            ts(w1[:], munion[:], EPS, None, op0=Alu.add)
            nc.vector.reciprocal(w1[:], w1[:])
            tt(miou[:], minter[:], w1[:], op=Alu.mult)
            tt(w1[:], mx1[:], nx1[:], op=Alu.min)
            tt(w2[:], mx2[:], nx2[:], op=Alu.max)
            tt(w2[:], w2[:], w1[:], op=Alu.subtract)  # ew
            tt(w1[:], my1[:], ny1[:], op=Alu.min)
            tt(w3[:], my2[:], ny2[:], op=Alu.max)
            tt(w3[:], w3[:], w1[:], op=Alu.subtract)  # eh
            menc = cp.tile([64, 1], F32)
            tt(menc[:], w2[:], w3[:], op=Alu.mult)
            tt(w1[:], menc[:], munion[:], op=Alu.subtract)
            ts(w2[:], menc[:], EPS, None, op0=Alu.add)
            nc.vector.reciprocal(w2[:], w2[:])
            tt(w1[:], w1[:], w2[:], op=Alu.mult)
            mgiou = cp.tile([64, 1], F32)
            tt(mgiou[:], miou[:], w1[:], op=Alu.subtract)
            ts(w4[:], mgiou[:], -1.0, 1.0, op0=Alu.mult, op1=Alu.add)  # 1-giou

            # per-sample sums: transpose each (64,1) vector and accumulate
            # rows 0 / 32 separately.
            sums3 = cp.tile([64, 3], F32)  # col 0=l1, 1=1-g, 2=po; rows 0/32
            for ci, vec in enumerate((l1p[:], w4[:], pom[:])):
                pkx = cp.tile([64, 32], F32, tag="pkx")
                nc.vector.memset(pkx[:], 0.0)
                nc.vector.tensor_copy(pkx[:, 0:1], vec)
                pkxT = cp.tile([64, 32], F32, tag="pkxT")
                nc.vector.transpose(pkxT[:], pkx[:])
                ts(pkxT[0:1, :], pkxT[0:1, :], 0.0, None, op0=Alu.add,
                   op1=Alu.add, accum_out=sums3[0:1, ci : ci + 1])
                ts(pkxT[32:33, :], pkxT[32:33, :], 0.0, None, op0=Alu.add,
                   op1=Alu.add, accum_out=sums3[32:33, ci : ci + 1])

            # objectness base: relu(po) + ln(1+exp(-|po|)) on the broadcast
            # po slab (seg 8); rows 0 / 32 give the per-sample rowsums.
            relu = cp.tile([64, N], F32)
            abspo = cp.tile([64, N], F32)
            sp = cp.tile([64, N], F32)
            basesum = cp.tile([64, 1], F32)
            ts(relu[:], seg(8), 0.0, None, op0=Alu.max)
            nc.scalar.activation(abspo[:], seg(8), Act.Abs)
            nc.scalar.activation(sp[:], abspo[:], Act.Exp, scale=-1.0)
            ts(sp[:], sp[:], 1.0, None, op0=Alu.add)
            nc.scalar.activation(sp[:], sp[:], Act.Ln)
            tt(relu[:], relu[:], sp[:], op=Alu.add)
            ts(relu[:], relu[:], 0.0, None, op0=Alu.add, op1=Alu.add,
               accum_out=basesum[:])

            # bbox_b = clip(l1sum/128 + clip(gsum/32, 0, 2), 0)
            # obj_b = clip((basesum - pomsum)/256, 0)
            # per-sample results at rows 0 and 32 of outsb cols 10/11.
            b1t = cp.tile([64, 1], F32)
            b2t = cp.tile([64, 1], F32)
            obt = cp.tile([64, 1], F32)
            for b in range(2):
                r = 32 * b
                bb = slice(r, r + 1)
                ts(b1t[bb], sums3[bb, 0:1], 1.0 / 128.0, None, op0=Alu.mult)
                ts(b2t[bb], sums3[bb, 1:2], 1.0 / 32.0, None, op0=Alu.mult)
                ts(b2t[bb], b2t[bb], 0.0, 2.0, op0=Alu.max, op1=Alu.min)
                tt(b1t[bb], b1t[bb], b2t[bb], op=Alu.add)
                ts(b1t[bb], b1t[bb], 0.0, None, op0=Alu.max)
                tt(obt[bb], basesum[bb], sums3[bb, 2:3], op=Alu.subtract)
                ts(obt[bb], obt[bb], 1.0 / 256.0, 0.0, op0=Alu.mult, op1=Alu.max)
                nc.vector.tensor_copy(outsb[bb, 10:11], b1t[bb])
                nc.vector.tensor_copy(outsb[bb, 11:12], obt[bb])

            nc.sync.dma_start(out[:], outsb[:])

    nc.compile()
    return nc


# ---------------- host side ----------------

def shard_inputs(pred_boxes, pred_objectness, caption_logits, gt_boxes, V8, NC=8):
    pbf = pred_boxes.astype(np.float32)
    x1n = np.minimum(pbf[..., 0], pbf[..., 2])
    y1n = np.minimum(pbf[..., 1], pbf[..., 3])
    x2n = np.maximum(pbf[..., 0], pbf[..., 2])
    y2n = np.maximum(pbf[..., 1], pbf[..., 3])
    rows = np.stack(
        [x1n, y1n, x2n, y2n, pbf[..., 0], pbf[..., 1], pbf[..., 2], pbf[..., 3],
         pred_objectness.astype(np.float32)], axis=1)  # (B, 9, N)
    pbig = np.broadcast_to(rows[:, None, :, :], (B, M, 9, N)).reshape(64, 9 * N)
    pbig = np.ascontiguousarray(pbig)
    po = np.ascontiguousarray(pred_objectness.reshape(B * N, 1).astype(np.float32))
    pb = np.ascontiguousarray(pred_boxes.reshape(B * N, 4).astype(np.float32))
    gb = np.ascontiguousarray(gt_boxes.reshape(B * M, 4).astype(np.float32))
    clv = caption_logits.reshape(B * N * L, NC, V8)
    in_maps = []
    for c in range(NC):
        in_maps.append({
            "cl": np.ascontiguousarray(clv[:, c, :]).astype(np.float32, copy=False),
            "pbig": pbig, "po": po, "pb": pb, "gb": gb,
        })
    return in_maps


def combine(results, caption_logits, gt_tokens, V8, NC=8):
    """results: list of per-core 'out' arrays (128,16)."""
    out0 = results[0]
    sums = np.zeros((GP, NBATCH), np.float64)
    for c in range(NC):
        sums += results[c][0:GP, 0:NBATCH].astype(np.float64)
    lse = np.log(sums)  # (120, 8): row p = k*30 + b*15 + l, col g; step = 4g+k
    lse_bsl = (
        lse.reshape(STEPS_PER_BATCH, B, LM1, NBATCH)
        .transpose(1, 3, 0, 2)
        .reshape(B, S, LM1)
    )
    pis = out0[0:64, 8].astype(np.int64).reshape(2, 32)
    gjs = out0[0:64, 9].astype(np.int64).reshape(2, 32)
    tok = np.asarray(gt_tokens).astype(np.int64)

    bidx = np.arange(B)[:, None, None]
    lidx = np.arange(LM1)[None, None, :]
    tgt = tok[bidx, gjs[:, :, None], lidx + 1]  # (B, S, LM1)
    tlog = caption_logits[bidx, pis[:, :, None], lidx, tgt].astype(np.float64)
    ce = (lse_bsl - tlog).mean(axis=2)  # (B, S)
    cap = np.clip(np.clip(ce, 0.0, None).mean(axis=1), 0.0, None)  # (B,)
    bbox = out0[[0, 32], 10].astype(np.float64)
    obj = out0[[0, 32], 11].astype(np.float64)
    total = max((5.0 * bbox + 0.1 * cap + obj).mean(), 0.0)
    comps = [5.0 * bbox.mean(), 0.1 * cap.mean(), obj.mean()]
    return np.array([total] + comps, np.float32)


# ---------------- entry points ----------------

V8_FULL = 4000
NC_CORES = 8
_CACHE = {}


def get_nc(V8=V8_FULL):
    key = V8
    if key not in _CACHE:
        _CACHE[key] = build_nc(V8, num_devices=NC_CORES)
    return _CACHE[key]


def run_device(in_maps, V8=V8_FULL, trace=False, **kw):
    from concourse.bass_utils import run_bass_kernel_spmd

    nc = get_nc(V8)
    return run_bass_kernel_spmd(
        nc, in_maps, core_ids=list(range(NC_CORES)), trace=trace, **kw)


def kernel(pred_boxes, pred_objectness, caption_logits, gt_boxes, gt_tokens):
    pred_boxes = np.asarray(pred_boxes, np.float32)
    pred_objectness = np.asarray(pred_objectness, np.float32)
    caption_logits = np.asarray(caption_logits, np.float32)
    gt_boxes = np.asarray(gt_boxes, np.float32)
    in_maps = shard_inputs(
        pred_boxes, pred_objectness, caption_logits, gt_boxes, V8_FULL, NC_CORES)
    res = run_device(in_maps)
    outs = [r["out"] for r in res.results]
    return combine(outs, caption_logits, gt_tokens, V8_FULL, NC_CORES)



# revision 2
# speedup vs baseline: 1.9425x; 1.9425x over previous
"""DetectionLoss Bass kernel for TRN2, 8-core SPMD (vocab-sharded).

Device (identical program on all 8 cores; only the vocab slice of
caption_logits differs):
  * build the (64,256) fused positive cost matrix ncf = 16 - cost (both
    samples stacked on the partition dim) from host-prepped box rows,
  * 32-step greedy matching entirely on the vector engine:
      - per-row top-1 via max/max_index over (64,256),
      - one (64,64) stream transpose moves per-row maxima and argmax+1
        into the free dim for both samples at partition rows 0/32,
      - global per-sample argmax + used-gt masking via an additive
        accumulator, used-pred masking by MULTIPLYING the (positive)
        cost column to zero with an iota compare (no register loads on
        the critical path),
  * per step, the matched predictions' caption-logit slabs (15 rows x
    V/8 floats each) are fetched with register-offset HWDGE DMAs -
    sample A issued from SP, sample B from ACT, so descriptor work
    stays off the vector engine and queues stay deep,
  * exp + free-dim accumulate on ACT every 4 steps -> per-(step-group,
    row) partial sum(exp) over this core's vocab slice.
Host: preps the broadcast box rows, shards caption_logits by vocab,
all-reduces the per-core partial sumexps, takes log, gathers target
token logits, and computes the scalar bbox/objectness losses and the
final weighted combination from the device-produced matching (pis,
gjs) - these are O(B*N) scalar reductions.
"""

import sys

sys.path.insert(0, "/opt/trn_rl_repo")

import numpy as np

import concourse.bacc as bacc
import concourse.mybir as mybir
from concourse.bass import ds
from concourse.tile import TileContext

F32 = mybir.dt.float32
I32 = mybir.dt.int32
U32 = mybir.dt.uint32
Alu = mybir.AluOpType
Act = mybir.ActivationFunctionType
Eng = mybir.EngineType

B, N, M, L = 2, 256, 32, 16
LM1 = L - 1  # 15 caption positions
S = M  # greedy steps
NEGBIG = -1.0e9
EPS = 1e-7
ROWS_PER_STEP = B * LM1  # 30 gathered rows per step
STEPS_PER_BATCH = 4
NBATCH = S // STEPS_PER_BATCH  # 8 ACT sweeps over (120, V8)
GP = STEPS_PER_BATCH * ROWS_PER_STEP  # 120
NSEG = 10  # pbig segments


def build_nc(V8: int, num_devices: int = 8):
    """Build the per-core Bass program. V8 = vocab slice width per core."""
    nc = bacc.Bacc(
        "TRN2", target_bir_lowering=False, debug=False, num_devices=num_devices
    )

    cl = nc.dram_tensor("cl", (B * N * L, V8), F32, kind="ExternalInput")
    # pbig: per (b,j) partition, NSEG x 256 row segments:
    # [x1n y1n x2n y2n x1 y1 x2 y2 sig14 a1]   (sig14 = sigmoid(po)+14)
    pbig = nc.dram_tensor("pbig", (64, NSEG * N), F32, kind="ExternalInput")
    # gbx: per (b,j) partition: [gx1n gy1n gx2n gy2n ga2 g0 g1 g2 g3]
    gbx = nc.dram_tensor("gbx", (64, 9), F32, kind="ExternalInput")

    outse = nc.dram_tensor("outse", (GP, NBATCH), F32, kind="ExternalOutput")
    pis_o = nc.dram_tensor("pis_o", (64, S), I32, kind="ExternalOutput")
    gjs_o = nc.dram_tensor("gjs_o", (64, 8 * S), U32, kind="ExternalOutput")

    # per-sample DRAM view for register-offset gathers
    cl2 = cl[:].rearrange("(b n l) v -> b n (l v)", b=B, n=N)  # (2, 256, L*V8)

    with TileContext(nc) as tc:
        with (
            tc.tile_pool(name="cpool", bufs=1) as cp,
            tc.tile_pool(name="gpool", bufs=3) as gp,
            tc.tile_pool(name="dpool", bufs=1) as dp,
        ):
            ts = nc.vector.tensor_scalar
            tt = nc.vector.tensor_tensor
            stt = nc.vector.scalar_tensor_tensor

            # ---------- input loads ----------
            pbig_sb = cp.tile([64, NSEG * N], F32)
            nc.sync.dma_start(pbig_sb[:], pbig[:])
            gbx_sb = cp.tile([64, 9], F32)
            nc.sync.dma_start(gbx_sb[:], gbx[:])

            def seg(k):
                return pbig_sb[:, k * N : (k + 1) * N]

            def gcol(k):
                return gbx_sb[:, k : k + 1]

            # constants
            iotap1 = cp.tile([64, N], F32)
            nc.gpsimd.iota(
                iotap1[:], pattern=[[1, N]], base=1, channel_multiplier=0,
                allow_small_or_imprecise_dtypes=True,
            )

            # ---------- cost matrix: ncf = 16 - cost  (all positive) ----------
            # 16 - cost = giou - l1 + sigmoid(po) + 14
            xi2 = cp.tile([64, N], F32)
            yi2 = cp.tile([64, N], F32)
            iw = cp.tile([64, N], F32)
            ih = cp.tile([64, N], F32)
            inter = cp.tile([64, N], F32)
            ts(xi2[:], seg(2), gcol(2), None, op0=Alu.min)
            stt(iw[:], seg(0), gcol(0), xi2[:], op0=Alu.max, op1=Alu.subtract)
            ts(iw[:], iw[:], -1.0, 0.0, op0=Alu.mult, op1=Alu.max)
            ts(yi2[:], seg(3), gcol(3), None, op0=Alu.min)
            stt(ih[:], seg(1), gcol(1), yi2[:], op0=Alu.max, op1=Alu.subtract)
            ts(ih[:], ih[:], -1.0, 0.0, op0=Alu.mult, op1=Alu.max)
            tt(inter[:], iw[:], ih[:], op=Alu.mult)

            ew = cp.tile([64, N], F32)
            eh = cp.tile([64, N], F32)
            enc = cp.tile([64, N], F32)
            ts(ew[:], seg(2), gcol(2), None, op0=Alu.max)
            stt(ew[:], seg(0), gcol(0), ew[:], op0=Alu.min, op1=Alu.subtract)
            ts(eh[:], seg(3), gcol(3), None, op0=Alu.max)
            stt(eh[:], seg(1), gcol(1), eh[:], op0=Alu.min, op1=Alu.subtract)
            tt(enc[:], ew[:], eh[:], op=Alu.mult)  # (-ew)*(-eh) = enc

            union = cp.tile([64, N], F32)
            stt(union[:], seg(9), gcol(4), inter[:], op0=Alu.add,
                op1=Alu.subtract)

            r1 = cp.tile([64, N], F32)
            r2 = cp.tile([64, N], F32)
            ts(r1[:], union[:], EPS, None, op0=Alu.add)
            nc.vector.reciprocal(r1[:], r1[:])
            ts(r2[:], enc[:], EPS, None, op0=Alu.add)
            nc.vector.reciprocal(r2[:], r2[:])

            giou = cp.tile([64, N], F32)
            tt(giou[:], inter[:], r1[:], op=Alu.mult)  # iou
            tt(enc[:], enc[:], union[:], op=Alu.subtract)  # enc - union
            tt(enc[:], enc[:], r2[:], op=Alu.mult)
            tt(giou[:], giou[:], enc[:], op=Alu.subtract)

            # l1 via strided diff tile + abs-reduce
            ld = cp.tile([64, 4 * N], F32)
            ldv = ld[:].rearrange("p (i c) -> p i c", c=4)
            for c in range(4):
                ts(ldv[:, :, c], seg(4 + c), gcol(5 + c), None,
                   op0=Alu.subtract)
            l1s = cp.tile([64, N], F32)
            nc.vector.tensor_reduce(
                l1s[:], ldv[:, :, :], axis=mybir.AxisListType.X, op=Alu.add,
                apply_absolute_value=True,
            )

            ncf = cp.tile([64, N], F32)
            stt(ncf[:], l1s[:], -1.0, giou[:], op0=Alu.mult, op1=Alu.add)
            tt(ncf[:], ncf[:], seg(8), op=Alu.add)

            # ---------- greedy matching ----------
            tile64 = cp.tile([64, 64], F32)
            nc.vector.memset(tile64[:], 0.0)
            T = cp.tile([64, 64], F32)
            gtm = cp.tile([64, 32], F32)
            nc.vector.memset(gtm[:], 0.0)
            ridx = cp.tile([64, 8], U32)
            tm = cp.tile([64, 32], F32)
            g8 = cp.tile([64, 8], F32)
            tsel = cp.tile([64, 32], F32)
            i1 = cp.tile([64, 8], F32)
            i1b = cp.tile([64, 1], F32)
            m = cp.tile([64, 32], F32)
            gjsall = cp.tile([64, 8 * S], U32)
            pisri = cp.tile([64, S], I32)
            outse_sb = cp.tile([GP, NBATCH], F32)

            for s in range(S):
                nc.vector.max(tile64[:, 0:8], ncf[:])
                nc.vector.max_index(ridx[:, 0:8], tile64[:, 0:8], ncf[:])
                ts(tile64[:, 32:33], ridx[:, 0:1], 1.0, None, op0=Alu.add)
                nc.vector.transpose(T[:], tile64[:])
                tt(tm[:], T[:, 0:32], gtm[:], op=Alu.add)
                nc.vector.max(g8[:], tm[:])
                stt(tsel[:], tm[:], g8[:, 0:1], T[:, 32:64],
                    op0=Alu.is_equal, op1=Alu.mult)
                nc.vector.max(i1[:], tsel[:])
                nc.vector.stream_shuffle(i1b[:], i1[:, 0:1], [0] * 32)
                stt(ncf[:], iotap1[:], i1b[:, 0:1], ncf[:],
                    op0=Alu.not_equal, op1=Alu.mult)
                # off-chain: gt mask update + outputs
                ts(m[:], tm[:], g8[:, 0:1], None, op0=Alu.is_equal)
                stt(gtm[:], m[:], NEGBIG, gtm[:], op0=Alu.mult, op1=Alu.add)
                nc.vector.max_index(gjsall[:, 8 * s : 8 * s + 8], g8[:], tm[:])
                ts(pisri[:, s : s + 1], i1[:, 0:1], -1.0, None, op0=Alu.add)

                # caption gathers: A from SP, B from ACT (two HWDGE rings)
                i0r = nc.values_load(
                    pisri[0:1, s : s + 1], engines=(Eng.SP,),
                    min_val=0, max_val=N - 1, skip_runtime_bounds_check=True)
                i1r = nc.values_load(
                    pisri[32:33, s : s + 1], engines=(Eng.Activation,),
                    min_val=0, max_val=N - 1, skip_runtime_bounds_check=True)
                g, k = divmod(s, STEPS_PER_BATCH)
                if k == 0:
                    gtile = gp.tile([GP, V8], F32, tag="gtile")
                base = k * ROWS_PER_STEP
                nc.sync.dma_start(
                    gtile[base : base + LM1, :],
                    cl2[0, ds(i0r, 1), 0 : LM1 * V8])
                nc.scalar.dma_start(
                    gtile[base + LM1 : base + ROWS_PER_STEP, :],
                    cl2[1, ds(i1r, 1), 0 : LM1 * V8])
                if k == STEPS_PER_BATCH - 1:
                    dump = dp.tile([GP, V8], F32, tag="dump")
                    nc.scalar.activation(
                        dump[:], gtile[:], Act.Exp,
                        accum_out=outse_sb[:, g : g + 1])

            # ---------- outputs ----------
            nc.sync.dma_start(outse[:], outse_sb[:])
            nc.sync.dma_start(pis_o[:], pisri[:])
            nc.sync.dma_start(gjs_o[:], gjsall[:])

    nc.compile()
    return nc


# ---------------- host side ----------------

def shard_inputs(pred_boxes, pred_objectness, caption_logits, gt_boxes, V8, NC=8):
    pbf = pred_boxes.astype(np.float32)
    po = pred_objectness.astype(np.float32)
    x1n = np.minimum(pbf[..., 0], pbf[..., 2])
    y1n = np.minimum(pbf[..., 1], pbf[..., 3])
    x2n = np.maximum(pbf[..., 0], pbf[..., 2])
    y2n = np.maximum(pbf[..., 1], pbf[..., 3])
    sig14 = (1.0 / (1.0 + np.exp(-po)) + 14.0).astype(np.float32)
    a1 = ((x2n - x1n) * (y2n - y1n)).astype(np.float32)
    rows = np.stack(
        [x1n, y1n, x2n, y2n, pbf[..., 0], pbf[..., 1], pbf[..., 2],
         pbf[..., 3], sig14, a1], axis=1)  # (B, NSEG, N)
    pbig = np.broadcast_to(rows[:, None, :, :], (B, M, NSEG, N)).reshape(
        64, NSEG * N)
    pbig = np.ascontiguousarray(pbig)

    gbf = gt_boxes.astype(np.float32)
    gx1n = np.minimum(gbf[..., 0], gbf[..., 2])
    gy1n = np.minimum(gbf[..., 1], gbf[..., 3])
    gx2n = np.maximum(gbf[..., 0], gbf[..., 2])
    gy2n = np.maximum(gbf[..., 1], gbf[..., 3])
    ga2 = (gx2n - gx1n) * (gy2n - gy1n)
    gbx = np.stack(
        [gx1n, gy1n, gx2n, gy2n, ga2, gbf[..., 0], gbf[..., 1], gbf[..., 2],
         gbf[..., 3]], axis=-1).reshape(64, 9).astype(np.float32)
    gbx = np.ascontiguousarray(gbx)

    clv = caption_logits.reshape(B * N * L, NC, V8)
    in_maps = []
    for c in range(NC):
        in_maps.append({
            "cl": np.ascontiguousarray(clv[:, c, :]).astype(np.float32, copy=False),
            "pbig": pbig, "gbx": gbx,
        })
    return in_maps


def _giou_np(b1, b2):
    def norm(b):
        x1 = np.minimum(b[..., 0], b[..., 2]); y1 = np.minimum(b[..., 1], b[..., 3])
        x2 = np.maximum(b[..., 0], b[..., 2]); y2 = np.maximum(b[..., 1], b[..., 3])
        return x1, y1, x2, y2
    ax1, ay1, ax2, ay2 = norm(b1)
    bx1, by1, bx2, by2 = norm(b2)
    xi1 = np.maximum(ax1, bx1); yi1 = np.maximum(ay1, by1)
    xi2 = np.minimum(ax2, bx2); yi2 = np.minimum(ay2, by2)
    inter = np.clip(xi2 - xi1, 0.0, None) * np.clip(yi2 - yi1, 0.0, None)
    a1 = (ax2 - ax1) * (ay2 - ay1)
    a2 = (bx2 - bx1) * (by2 - by1)
    union = a1 + a2 - inter
    iou = inter / (union + EPS)
    xe1 = np.minimum(ax1, bx1); ye1 = np.minimum(ay1, by1)
    xe2 = np.maximum(ax2, bx2); ye2 = np.maximum(ay2, by2)
    enc = (xe2 - xe1) * (ye2 - ye1)
    return iou - (enc - union) / (enc + EPS)


def combine(results, caption_logits, gt_tokens, pred_boxes, pred_objectness,
            gt_boxes, V8, NC=8):
    """results: list of per-core dicts with outse/pis_o/gjs_o."""
    out0 = results[0]
    sums = np.zeros((GP, NBATCH), np.float64)
    for c in range(NC):
        sums += results[c]["outse"].astype(np.float64)
    lse = np.log(sums)  # (120, 8): row p = k*30 + b*15 + l, col g; step = 4g+k
    lse_bsl = (
        lse.reshape(STEPS_PER_BATCH, B, LM1, NBATCH)
        .transpose(1, 3, 0, 2)
        .reshape(B, S, LM1)
    )
    pis = out0["pis_o"][[0, 32], :].astype(np.int64)  # (2, 32)
    gjs = out0["gjs_o"][[0, 32], ::8].astype(np.int64)  # (2, 32)
    tok = np.asarray(gt_tokens).astype(np.int64)

    bidx = np.arange(B)[:, None, None]
    lidx = np.arange(LM1)[None, None, :]
    tgt = tok[bidx, gjs[:, :, None], lidx + 1]  # (B, S, LM1)
    tlog = caption_logits[bidx, pis[:, :, None], lidx, tgt].astype(np.float64)
    ce = (lse_bsl - tlog).mean(axis=2)  # (B, S)
    cap = np.clip(np.clip(ce, 0.0, None).mean(axis=1), 0.0, None)  # (B,)

    pb = np.asarray(pred_boxes, np.float64)
    gb = np.asarray(gt_boxes, np.float64)
    po = np.asarray(pred_objectness, np.float64)
    bbox = np.zeros(B); obj = np.zeros(B)
    for b in range(B):
        mp = pb[b][pis[b]]; mg = gb[b][gjs[b]]
        l1_loss = np.abs(mp - mg).mean()
        giou_loss = np.clip((1.0 - _giou_np(mp, mg)).mean(), 0.0, 2.0)
        bbox[b] = max(l1_loss + giou_loss, 0.0)
        t = np.zeros(N); t[pis[b]] = 1.0
        ob = (np.maximum(po[b], 0.0) - po[b] * t
              + np.log1p(np.exp(-np.abs(po[b])))).mean()
        obj[b] = max(ob, 0.0)

    total = max((5.0 * bbox + 0.1 * cap + obj).mean(), 0.0)
    comps = [5.0 * bbox.mean(), 0.1 * cap.mean(), obj.mean()]
    return np.array([total] + comps, np.float32)


# ---------------- entry points ----------------

V8_FULL = 4000
NC_CORES = 8
_CACHE = {}


def get_nc(V8=V8_FULL):
    key = V8
    if key not in _CACHE:
        _CACHE[key] = build_nc(V8, num_devices=NC_CORES)
    return _CACHE[key]


def run_device(in_maps, V8=V8_FULL, trace=False, **kw):
    from concourse.bass_utils import run_bass_kernel_spmd

    nc = get_nc(V8)
    return run_bass_kernel_spmd(
        nc, in_maps, core_ids=list(range(NC_CORES)), trace=trace, **kw)


def kernel(pred_boxes, pred_objectness, caption_logits, gt_boxes, gt_tokens):
    pred_boxes = np.asarray(pred_boxes, np.float32)
    pred_objectness = np.asarray(pred_objectness, np.float32)
    caption_logits = np.asarray(caption_logits, np.float32)
    gt_boxes = np.asarray(gt_boxes, np.float32)
    in_maps = shard_inputs(
        pred_boxes, pred_objectness, caption_logits, gt_boxes, V8_FULL, NC_CORES)
    res = run_device(in_maps)
    return combine(res.results, caption_logits, gt_tokens, pred_boxes,
                   pred_objectness, gt_boxes, V8_FULL, NC_CORES)
